# revision 6
# baseline (speedup 1.0000x reference)
"""AdaptiveAntiAlias Trainium2 kernel.

out = 0.6 * gaussian5x5_zeropad(images) + 0.4 * bilateral5x5_reflect(images)

Data-parallel over the batch dim: 8 images -> 8 NeuronCores, one (3,512,512)
image per core.

Per-core layout: each channel's 512 rows are split over 128 SBUF partitions
(4 rows each). Every partition holds its 4 output rows plus a 2-row halo on
each side of the *padded* (516-wide) image, so all 25 stencil taps are plain
free-dim offset views of one [128, 8, 516] tile.
"""

import math

import numpy as np

import bass_rust
import concourse.bacc as bacc
import concourse.mybir as mybir
import concourse.tile as tile
from concourse.bass_utils import run_bass_kernel_spmd

F32 = mybir.dt.float32
BF16 = mybir.dt.bfloat16
AL = mybir.AluOpType
AF = mybir.ActivationFunctionType

N_CORES = 8
C, H, W = 3, 512, 512
PADW = W + 4          # 516
R = 4                 # output rows per partition
P = 128               # partitions

GX = [math.exp(-((i - 2) ** 2) / 2.0) for i in range(5)]   # spatial 1-D kernel
S1 = sum(GX)

_NC_CACHE = {}


def _overlap_view(ap, offset_elems, pairs):
    """Return a copy of `ap` with a manually constructed (possibly
    overlapping) access pattern. `pairs` is [[step, count], ...]."""
    v = ap.copy()
    v.offset = v.offset + offset_elems
    v.ap = bass_rust.VecI64Pair(pairs)
    return v


def _load_reflect_tile(nc, pr, x, c):
    """Fill SBUF tile pr[P, 8, 516] with the reflect-padded channel c:
    partition p row i col j == rpad[4p + i, j] of the 516x516 reflect-pad.
    Rows come contiguous from DRAM; the 4 pad columns are fixed up with
    tiny on-chip copies afterwards (jnp 'reflect': col -1 -> col 1, ...)."""
    xc = x[c]
    # partition 0: rows i=2..7 <- image rows 0..5; halo rows 0,1 <- rows 2,1
    nc.sync.dma_start(out=pr[0:1, 2:8, 2:514], in_=xc[0:6, :].unsqueeze(0))
    nc.sync.dma_start(out=pr[0:1, 0:1, 2:514], in_=xc[2:3, :].unsqueeze(0))
    nc.sync.dma_start(out=pr[0:1, 1:2, 2:514], in_=xc[1:2, :].unsqueeze(0))
    # partitions 1..126: rows i=0..7 <- image rows 4p-2 .. 4p+5 (overlapping)
    src = _overlap_view(xc, (4 * 1 - 2) * W, [[4 * W, 126], [W, 8], [1, W]])
    nc.sync.dma_start(out=pr[1:127, :, 2:514], in_=src)
    # partition 127: rows i=0..5 <- rows 506..511; halo rows 6,7 <- 510,509
    nc.sync.dma_start(out=pr[127:128, 0:6, 2:514], in_=xc[506:512, :].unsqueeze(0))
    nc.sync.dma_start(out=pr[127:128, 6:7, 2:514], in_=xc[510:511, :].unsqueeze(0))
    nc.sync.dma_start(out=pr[127:128, 7:8, 2:514], in_=xc[509:510, :].unsqueeze(0))
    # pad columns: 0 <- 4, 1 <- 3, 514 <- 512, 515 <- 511 (in padded coords)
    nc.vector.tensor_copy(pr[:, :, 0:1], pr[:, :, 4:5])
    nc.vector.tensor_copy(pr[:, :, 1:2], pr[:, :, 3:4])
    nc.vector.tensor_copy(pr[:, :, 514:515], pr[:, :, 512:513])
    nc.vector.tensor_copy(pr[:, :, 515:516], pr[:, :, 511:512])


def _load_zero_pad_tile(nc, pz, x, c):
    """Fill SBUF tile pz[P, 8, 516] with the zero-padded channel c, such that
    partition p row i col j == zpad[4p + i, j] of the 516x516 zero-pad."""
    nc.gpsimd.memset(pz[:, :, :], 0.0)
    xc = x[c]
    # partition 0: rows i=2..7 <- image rows 0..5
    nc.sync.dma_start(out=pz[0:1, 2:8, 2:514], in_=xc[0:6, :].unsqueeze(0))
    # partitions 1..126: rows i=0..7 <- image rows 4p-2 .. 4p+5 (overlapping)
    src = _overlap_view(xc, (4 * 1 - 2) * W, [[4 * W, 126], [W, 8], [1, W]])
    nc.sync.dma_start(out=pz[1:127, :, 2:514], in_=src)
    # partition 127: rows i=0..5 <- image rows 506..511
    nc.sync.dma_start(out=pz[127:128, 0:6, 2:514], in_=xc[506:512, :].unsqueeze(0))


def build_nc():
    nc = bacc.Bacc(
        "TRN2", target_bir_lowering=False, debug=False, num_devices=N_CORES
    )
    x = nc.dram_tensor("images", [C, H, W], F32, kind="ExternalInput").ap()
    y = nc.dram_tensor("out", [C, H, W], F32, kind="ExternalOutput").ap()

    sqrt50 = math.sqrt(50.0)

    with tile.TileContext(nc) as tc:
        with (
            tc.tile_pool(name="pads", bufs=2) as pads,
            tc.tile_pool(name="zpads", bufs=1) as zpads,
            tc.tile_pool(name="work", bufs=3) as work,
            tc.tile_pool(name="accs", bufs=1) as accs,
            tc.tile_pool(name="gtmp", bufs=1) as gtmp,
        ):
            for c in range(C):
                pr = pads.tile([P, 8, PADW], F32, tag="pr")
                _load_reflect_tile(nc, pr, x, c)
                pz = zpads.tile([P, 8, PADW], F32, tag="pz")
                _load_zero_pad_tile(nc, pz, x, c)

                # ---- separable gaussian on the zero-padded tile ----
                gh = gtmp.tile([P, 8, W], F32, tag="gh")
                nc.vector.tensor_scalar_mul(gh[:], pz[:, :, 0:W], GX[0])
                for dj in range(1, 5):
                    nc.vector.scalar_tensor_tensor(
                        gh[:], in0=pz[:, :, dj:dj + W], scalar=GX[dj],
                        in1=gh[:], op0=AL.mult, op1=AL.add,
                    )
                gv = accs.tile([P, R, W], F32, tag="gv")
                vs0 = GX[0] * 0.6 / (S1 * S1)
                nc.vector.tensor_scalar_mul(gv[:], gh[:, 0:R, :], vs0)
                for di in range(1, 5):
                    vs = GX[di] * 0.6 / (S1 * S1)
                    nc.vector.scalar_tensor_tensor(
                        gv[:], in0=gh[:, di:di + R, :], scalar=vs,
                        in1=gv[:], op0=AL.mult, op1=AL.add,
                    )

                # ---- bilateral on the reflect-padded tile ----
                ctr = pr[:, 2:2 + R, 2:2 + W]
                accw = accs.tile([P, R, W], F32, tag="accw")
                acct = accs.tile([P, R, W], F32, tag="acct")

                taps = [(di, dj) for di in range(5) for dj in range(5)
                        if not (di == 2 and dj == 2)]
                first = True
                for di, dj in taps:
                    pv = pr[:, di:di + R, dj:dj + W]
                    sw = math.exp(-((di - 2) ** 2 + (dj - 2) ** 2) / 2.0)
                    d = work.tile([P, R, W], F32, tag="d")
                    nc.vector.tensor_tensor(d[:], pv, ctr, AL.subtract)
                    s = work.tile([P, R, W], F32, tag="s")
                    nc.scalar.activation(s[:], d[:], AF.Square, scale=sqrt50)
                    e = work.tile([P, R, W], F32, tag="e")
                    nc.scalar.activation(e[:], s[:], AF.Exp, scale=-1.0)
                    t = work.tile([P, R, W], F32, tag="t")
                    nc.vector.tensor_tensor(t[:], e[:], pv, AL.mult)
                    if first:
                        # fold in the center tap: e_center = 1, t_center = ctr
                        nc.vector.tensor_scalar(accw[:], e[:], sw, 1.0,
                                                AL.mult, AL.add)
                        nc.vector.scalar_tensor_tensor(
                            acct[:], in0=t[:], scalar=sw, in1=ctr,
                            op0=AL.mult, op1=AL.add)
                        first = False
                    else:
                        nc.vector.scalar_tensor_tensor(
                            accw[:], in0=e[:], scalar=sw, in1=accw[:],
                            op0=AL.mult, op1=AL.add)
                        nc.vector.scalar_tensor_tensor(
                            acct[:], in0=t[:], scalar=sw, in1=acct[:],
                            op0=AL.mult, op1=AL.add)

                # ---- combine: out = 0.4 * acct / accw + gv ----
                r = work.tile([P, R, W], F32, tag="d")
                nc.vector.reciprocal_approx_fast(r[:], accw[:])
                b = work.tile([P, R, W], F32, tag="s")
                nc.vector.tensor_tensor(b[:], acct[:], r[:], AL.mult)
                o = work.tile([P, R, W], F32, tag="e")
                nc.vector.scalar_tensor_tensor(
                    o[:], in0=b[:], scalar=0.4, in1=gv[:],
                    op0=AL.mult, op1=AL.add,
                )
                ydst = y[c].rearrange("(p r) w -> p r w", r=R)
                nc.sync.dma_start(out=ydst, in_=o[:])

    nc.compile()
    return nc


def _get_nc():
    if "nc" not in _NC_CACHE:
        _NC_CACHE["nc"] = build_nc()
    return _NC_CACHE["nc"]


def kernel(images: np.ndarray) -> np.ndarray:
    images = np.ascontiguousarray(np.asarray(images, dtype=np.float32))
    B = images.shape[0]
    assert images.shape == (B, C, H, W) and B == N_CORES
    nc = _get_nc()
    in_maps = [{"images": images[i]} for i in range(N_CORES)]
    res = run_bass_kernel_spmd(nc, in_maps, core_ids=list(range(N_CORES)))
    return np.stack([res.results[i]["out"] for i in range(N_CORES)], axis=0)


# revision 8
# speedup vs baseline: 2.4008x; 2.4008x over previous
"""AdaptiveAntiAlias Trainium2 kernel.

out = 0.6 * gaussian5x5_zeropad(images) + 0.4 * bilateral5x5_reflect(images)

Data-parallel over the batch dim: 8 images -> 8 NeuronCores, one (3,512,512)
image per core.

Per-core layout: each channel's 512 rows are split over 128 SBUF partitions
(4 rows each). Every partition holds its 4 output rows plus a 2-row halo on
each side of the *padded* (516-wide) image, so all 25 stencil taps are plain
free-dim offset views of one [128, 8, 516] tile.

Engine split per bilateral tap (24 non-center taps):
  VectorE : d = p - c, t = e * p            (bf16, 2x packed mode; an even-
            and an odd-column copy of the padded tile keep every tap 4B
            aligned)
  ScalarE : e = Derivative_Erf(sqrt(50) d) = 2/sqrt(pi) * exp(-50 d^2)
  TensorE : acct += sw * t, accw += sw * e  via scaled-identity matmuls
            accumulating into PSUM (sw absorbs the 2/sqrt(pi)).
The separable 5x5 gaussian runs on VectorE with fused scalar_tensor_tensor
multiply-adds over a zero-padded copy.
"""

import math

import numpy as np
import ml_dtypes

import bass_rust
import concourse.bacc as bacc
import concourse.mybir as mybir
import concourse.tile as tile
from concourse.bass_utils import run_bass_kernel_spmd

F32 = mybir.dt.float32
BF16 = mybir.dt.bfloat16
AL = mybir.AluOpType
AF = mybir.ActivationFunctionType

N_CORES = 8
C, H, W = 3, 512, 512
PADW = W + 4          # 516
R = 4                 # output rows per partition
P = 128               # partitions

GX = [math.exp(-((i - 2) ** 2) / 2.0) for i in range(5)]   # spatial 1-D kernel
S1 = sum(GX)
C_ERF = math.sqrt(math.pi) / 2.0     # Derivative_Erf carries 2/sqrt(pi)

# identity scales: slot 0 = 1.0 (center/ones), slots 1.. = sw * C_ERF per
# distinct (di-2)^2+(dj-2)^2
_S2S = [1, 2, 4, 5, 8]
_ID_SCALE = [1.0] + [math.exp(-s2 / 2.0) * C_ERF for s2 in _S2S]
N_ID = len(_ID_SCALE)

_NC_CACHE = {}


def _identities() -> np.ndarray:
    out = np.zeros((P, N_ID * P), dtype=ml_dtypes.bfloat16)
    for j, sc in enumerate(_ID_SCALE):
        out[:, j * P:(j + 1) * P] = (np.eye(P) * sc).astype(ml_dtypes.bfloat16)
    return out


def _overlap_view(ap, offset_elems, pairs):
    """Copy of `ap` with a manually constructed (possibly overlapping)
    access pattern; `pairs` is [[step, count], ...]."""
    v = ap.copy()
    v.offset = v.offset + offset_elems
    v.ap = bass_rust.VecI64Pair(pairs)
    return v


def _load_reflect_tile(nc, pr, x, c):
    """Fill SBUF tile pr[P, 8, 516] with the reflect-padded channel c:
    partition p row i col j == rpad[4p + i, j] of the 516x516 reflect-pad."""
    xc = x[c]
    nc.sync.dma_start(out=pr[0:1, 2:8, 2:514], in_=xc[0:6, :].unsqueeze(0))
    nc.sync.dma_start(out=pr[0:1, 0:1, 2:514], in_=xc[2:3, :].unsqueeze(0))
    nc.sync.dma_start(out=pr[0:1, 1:2, 2:514], in_=xc[1:2, :].unsqueeze(0))
    src = _overlap_view(xc, (4 * 1 - 2) * W, [[4 * W, 126], [W, 8], [1, W]])
    nc.sync.dma_start(out=pr[1:127, :, 2:514], in_=src)
    nc.sync.dma_start(out=pr[127:128, 0:6, 2:514], in_=xc[506:512, :].unsqueeze(0))
    nc.sync.dma_start(out=pr[127:128, 6:7, 2:514], in_=xc[510:511, :].unsqueeze(0))
    nc.sync.dma_start(out=pr[127:128, 7:8, 2:514], in_=xc[509:510, :].unsqueeze(0))
    # pad columns: 0 <- 4, 1 <- 3, 514 <- 512, 515 <- 511 (in padded coords)
    nc.vector.tensor_copy(pr[:, :, 0:1], pr[:, :, 4:5])
    nc.vector.tensor_copy(pr[:, :, 1:2], pr[:, :, 3:4])
    nc.vector.tensor_copy(pr[:, :, 514:515], pr[:, :, 512:513])
    nc.vector.tensor_copy(pr[:, :, 515:516], pr[:, :, 511:512])


def _load_zero_pad_tile(nc, pz, x, c):
    """SBUF tile pz[P, 8, 516] = zero-padded channel c (same indexing)."""
    nc.gpsimd.memset(pz[:, :, :], 0.0)
    xc = x[c]
    nc.sync.dma_start(out=pz[0:1, 2:8, 2:514], in_=xc[0:6, :].unsqueeze(0))
    src = _overlap_view(xc, (4 * 1 - 2) * W, [[4 * W, 126], [W, 8], [1, W]])
    nc.sync.dma_start(out=pz[1:127, :, 2:514], in_=src)
    nc.sync.dma_start(out=pz[127:128, 0:6, 2:514], in_=xc[506:512, :].unsqueeze(0))


def build_nc():
    nc = bacc.Bacc(
        "TRN2", target_bir_lowering=False, debug=False, num_devices=N_CORES
    )
    x = nc.dram_tensor("images", [C, H, W], F32, kind="ExternalInput").ap()
    idents = nc.dram_tensor("idents", [P, N_ID * P], BF16,
                            kind="ExternalInput").ap()
    y = nc.dram_tensor("out", [C, H, W], F32, kind="ExternalOutput").ap()

    sqrt50 = math.sqrt(50.0)

    with tile.TileContext(nc) as tc:
        with (
            tc.tile_pool(name="const", bufs=1) as constp,
            tc.tile_pool(name="pads", bufs=2) as pads,
            tc.tile_pool(name="bpads", bufs=2) as bpads,
            tc.tile_pool(name="zpads", bufs=1) as zpads,
            tc.tile_pool(name="work", bufs=3) as work,
            tc.tile_pool(name="fin", bufs=1) as fin,
            tc.tile_pool(name="accs", bufs=1) as accs,
            tc.tile_pool(name="gtmp", bufs=1) as gtmp,
            tc.tile_pool(name="psum", bufs=1, space="PSUM") as psum,
        ):
            idt = constp.tile([P, N_ID * P], BF16, tag="idt")
            nc.sync.dma_start(out=idt[:], in_=idents)
            ones_bf = constp.tile([P, R, W], BF16, tag="ones")
            nc.gpsimd.memset(ones_bf[:], 1.0)

            def ident(j):
                return idt[:, j * P:(j + 1) * P]

            for c in range(C):
                pr = pads.tile([P, 8, PADW], F32, tag="pr")
                _load_reflect_tile(nc, pr, x, c)
                # bf16 copies: even-aligned and odd-aligned (shifted 1 col)
                pr_ev = bpads.tile([P, 8, PADW], BF16, tag="pr_ev")
                nc.vector.tensor_copy(pr_ev[:], pr[:])
                pr_od = bpads.tile([P, 8, PADW - 2], BF16, tag="pr_od")
                nc.vector.tensor_copy(pr_od[:], pr[:, :, 1:PADW - 1])

                pz = zpads.tile([P, 8, PADW], F32, tag="pz")
                _load_zero_pad_tile(nc, pz, x, c)

                # ---- separable gaussian (fp32, zero padding) ----
                gh = gtmp.tile([P, 8, W], F32, tag="gh")
                nc.vector.tensor_scalar_mul(gh[:], pz[:, :, 0:W], GX[0])
                for dj in range(1, 5):
                    nc.vector.scalar_tensor_tensor(
                        gh[:], in0=pz[:, :, dj:dj + W], scalar=GX[dj],
                        in1=gh[:], op0=AL.mult, op1=AL.add,
                    )
                gv = accs.tile([P, R, W], F32, tag="gv")
                vs0 = GX[0] * 0.6 / (S1 * S1)
                nc.vector.tensor_scalar_mul(gv[:], gh[:, 0:R, :], vs0)
                for di in range(1, 5):
                    vs = GX[di] * 0.6 / (S1 * S1)
                    nc.vector.scalar_tensor_tensor(
                        gv[:], in0=gh[:, di:di + R, :], scalar=vs,
                        in1=gv[:], op0=AL.mult, op1=AL.add,
                    )

                # ---- bilateral: accumulate in PSUM via TensorE ----
                ctr = pr_ev[:, 2:2 + R, 2:2 + W]          # bf16 center
                acct_p = psum.tile([P, R, W], F32, tag="acct")
                accw_p = psum.tile([P, R, W], F32, tag="accw")
                # center tap: acct = ctr, accw = 1
                for n in range(R):
                    nc.tensor.matmul(acct_p[:, n, :], lhsT=ident(0),
                                     rhs=ctr[:, n, :], start=True, stop=False)
                    nc.tensor.matmul(accw_p[:, n, :], lhsT=ident(0),
                                     rhs=ones_bf[:, n, :], start=True,
                                     stop=False)

                taps = [(di, dj) for di in range(5) for dj in range(5)
                        if not (di == 2 and dj == 2)]
                for ti, (di, dj) in enumerate(taps):
                    last = ti == len(taps) - 1
                    if dj % 2 == 0:
                        pv = pr_ev[:, di:di + R, dj:dj + W]
                    else:
                        pv = pr_od[:, di:di + R, dj - 1:dj - 1 + W]
                    jid = 1 + _S2S.index((di - 2) ** 2 + (dj - 2) ** 2)
                    d = work.tile([P, R, W], BF16, tag="d")
                    nc.vector.tensor_tensor(d[:], pv, ctr, AL.subtract)
                    e = work.tile([P, R, W], BF16, tag="e")
                    nc.scalar.activation(e[:], d[:], AF.Derivative_Erf,
                                         scale=sqrt50)
                    t = work.tile([P, R, W], BF16, tag="t")
                    nc.vector.tensor_tensor(t[:], e[:], pv, AL.mult)
                    for n in range(R):
                        nc.tensor.matmul(acct_p[:, n, :], lhsT=ident(jid),
                                         rhs=t[:, n, :], start=False,
                                         stop=last)
                        nc.tensor.matmul(accw_p[:, n, :], lhsT=ident(jid),
                                         rhs=e[:, n, :], start=False,
                                         stop=last)

                # ---- combine: out = 0.4 * acct / accw + gv ----
                wsum = accs.tile([P, R, W], F32, tag="wsum")
                nc.scalar.activation(wsum[:], accw_p[:], AF.Copy)
                tnum = accs.tile([P, R, W], F32, tag="tnum")
                nc.scalar.activation(tnum[:], acct_p[:], AF.Copy)
                r = fin.tile([P, R, W], F32, tag="r")
                nc.vector.reciprocal_approx_fast(r[:], wsum[:])
                b = fin.tile([P, R, W], F32, tag="b")
                nc.vector.tensor_tensor(b[:], tnum[:], r[:], AL.mult)
                o = fin.tile([P, R, W], F32, tag="o")
                nc.vector.scalar_tensor_tensor(
                    o[:], in0=b[:], scalar=0.4, in1=gv[:],
                    op0=AL.mult, op1=AL.add,
                )
                ydst = y[c].rearrange("(p r) w -> p r w", r=R)
                nc.sync.dma_start(out=ydst, in_=o[:])

    nc.compile()
    return nc


def _get_nc():
    if "nc" not in _NC_CACHE:
        _NC_CACHE["nc"] = build_nc()
    return _NC_CACHE["nc"]


def _in_maps(images):
    idn = _identities()
    return [{"images": images[i], "idents": idn} for i in range(N_CORES)]


def kernel(images: np.ndarray) -> np.ndarray:
    images = np.ascontiguousarray(np.asarray(images, dtype=np.float32))
    B = images.shape[0]
    assert images.shape == (B, C, H, W) and B == N_CORES
    nc = _get_nc()
    res = run_bass_kernel_spmd(nc, _in_maps(images),
                               core_ids=list(range(N_CORES)))
    return np.stack([res.results[i]["out"] for i in range(N_CORES)], axis=0)
